# revision 33
# baseline (speedup 1.0000x reference)
"""Trainium2 Bass kernel for the fused attention+LN+GELU+projection module.

Shapes (hardcoded): x [B=256, S=512, D=512]; k/q/v_w [H=256, D]; attn_bias [S, H];
out_w [D, S*H]; output [B, 1, D].

Distribution across 8 NeuronCores:
 - attention/LN/GELU: data-parallel over batch, 32 batches/core, bf16 matmuls.
 - output projection: contraction dim S*H sharded 8 ways. Activations are
   redistributed batch-sharded -> contraction-sharded by THREE chunked
   AllToAlls (batches 0-15, 16-23, 24-31); the first two and their matmul
   passes overlap with the back half of attention. Received [b, sh] blocks
   are transposed to [sh, b] by XBAR DMA-transpose (no PE work), multiplied
   against out_w^T tiles (partially SBUF-resident), partials AllReduce-summed.
"""

import sys

sys.path.insert(0, "/opt/trn_rl_repo")

import numpy as np
import ml_dtypes

import concourse.bacc as bacc
import concourse.tile as tile
from concourse import mybir
from concourse.bass_utils import run_bass_kernel_spmd
from concourse.hw_specs import get_activation_tables
from concourse.tile_rust import add_dep_helper
import bass_rust as _bass_rust

N_CORES = 8
B, S, H, D = 256, 512, 256, 512
NB = B // N_CORES          # batches per core (32)
CHUNKS = [(0, 12), (12, 12), (24, 8)]  # (start, len) of A2A chunks
SCALE = 1.0 / (B ** 0.5)   # score scale (batch-size based, faithful to ref)
LN_EPS = 1e-5
NDT = D // 128             # 4 d-tiles
NST = S // 128             # 4 s-tiles
NHT = H // 128             # 2 h-tiles
SLICE = (S // N_CORES) * H  # 16384 contraction elems per core
NC_T = SLICE // 128        # 128 contraction tiles per core
G = 4                      # ACT-table batch group size
P8G = 16                   # contraction tiles per phase-8 group
NP8G = NC_T // P8G         # 8 groups per pass
RES_G = 4                  # ow groups kept SBUF-resident (rest streamed)
XT_PRE = 8                 # xt loads issued this many batches ahead
GPS_TT = False             # run softmax-normalize mul/add on gpsimd

F32 = mybir.dt.float32
BF16 = mybir.dt.bfloat16
AF = mybir.ActivationFunctionType
BBF16 = ml_dtypes.bfloat16


class _Bacc(bacc.Bacc):
    """Bacc whose activation-table binding is restricted so that exp/ln are
    only servable by natural_log_exp_and_others and gelu by gelu_and_others.
    Avoids per-op ACT_TABLE_LOAD thrash (~2.7us each) from the default
    first-match binding. Table ids keep their act_info.json order."""

    def insert_act_table_loads(self):
        has_activation = any(
            isinstance(i, mybir.InstActivation)
            for b in self.main_func.blocks
            for i in b.instructions
        )
        if not has_activation:
            return
        keep = {"natural_log_exp_and_others", "gelu_and_others"}
        strip = {AF.Exp, AF.Ln, AF.Gelu}
        tables = []
        for name, funcs in get_activation_tables(self.m.arch).items():
            if name not in keep:
                funcs = funcs - strip
            tables.append((name, funcs))
        _bass_rust.insert_act_table_loads(self, tables)


def _chunk_of(b):
    for c, (lo, ln) in enumerate(CHUNKS):
        if lo <= b < lo + ln:
            return c, b - lo
    raise AssertionError


def _build(ln_trivial: bool):
    nc = _Bacc("TRN2", target_bir_lowering=False, debug=False,
               num_devices=N_CORES)

    # ---- DRAM I/O ----
    xT = nc.dram_tensor("xT", [NB, NDT, 128, S], BF16, kind="ExternalInput").ap()
    kq_wT = nc.dram_tensor("kq_wT", [NDT, 128, 2 * H], BF16, kind="ExternalInput").ap()
    v_wT = nc.dram_tensor("v_wT", [NDT, 128, H], BF16, kind="ExternalInput").ap()
    kq_b = nc.dram_tensor("kq_b", [128, 2 * H], F32, kind="ExternalInput").ap()
    v_b2 = nc.dram_tensor("v_b2", [NHT, 128, 1], F32, kind="ExternalInput").ap()
    ab = nc.dram_tensor("ab", [NST, 128, H], BF16, kind="ExternalInput").ap()
    outb8 = nc.dram_tensor("outb8", [128, D], F32, kind="ExternalInput").ap()
    ones_b = nc.dram_tensor("ones_b", [128, 128], BF16, kind="ExternalInput").ap()
    owT = nc.dram_tensor("owT", [NC_T, 128, D], BF16, kind="ExternalInput").ap()
    if not ln_trivial:
        lng = nc.dram_tensor("lng", [128, H], F32, kind="ExternalInput").ap()
        lnb = nc.dram_tensor("lnb", [128, H], F32, kind="ExternalInput").ap()
    y_out = nc.dram_tensor("y", [N_CORES * NB, D], F32, kind="ExternalOutput").ap()

    # internal DRAM (collective buffers); chunked: [dst_core, local_b, s, h]
    a2a_in = [nc.dram_tensor(f"a2a_in{c}", [N_CORES, ln, S // N_CORES, H],
                             BF16).ap() for c, (lo, ln) in enumerate(CHUNKS)]
    a2a_out = [nc.dram_tensor(f"a2a_out{c}", [N_CORES, ln, S // N_CORES, H],
                              BF16).ap() for c, (lo, ln) in enumerate(CHUNKS)]
    y_bounce = [nc.dram_tensor(f"y_bounce{c}", [N_CORES, ln, D], F32).ap()
                for c, (lo, ln) in enumerate(CHUNKS)]
    y_red = [nc.dram_tensor(f"y_red{c}", [N_CORES, ln, D], F32,
                            addr_space="Shared").ap()
             for c, (lo, ln) in enumerate(CHUNKS)]

    from contextlib import ExitStack
    with ExitStack() as _stk:
        tc = _stk.enter_context(tile.TileContext(nc))

        def _pool(name, bufs, space="SBUF"):
            return _stk.enter_context(
                tc.tile_pool(name=name, bufs=bufs, space=space))

        constp = _pool("const", 1)
        owresp = _pool("owres", 1)
        xtp = _pool("xt", XT_PRE + 1)
        kqp = _pool("kqsb", 6)
        vtp = _pool("vtsb", 4)
        ep = _pool("esb", 4)
        bcp = _pool("bcsb", 2)
        t1p = _pool("t1sb", 4)
        tp = _pool("tsb", 25)
        actp = _pool("actsb", 5)
        statp = _pool("stat", 16)
        lnstatp = _pool("lnstat", 40)
        at0p = _pool("at0sb", 3)
        at1p = _pool("at1sb", 2)
        owstrp = _pool("owstr", 2)
        ysbp = _pool("ysb", 2)
        if True:
            # batch 0's input and the projection weights lead the sync queue
            xt_tiles = {}
            t0 = xtp.tile([128, NDT, S], BF16, tag="xt", name="xtp0")
            nc.sync.dma_start(t0[:], xT[0].transpose([1, 0, 2]))
            xt_tiles[0] = t0
            kqw_sb = constp.tile([128, NDT, 2 * H], BF16, tag="kqw")
            nc.sync.dma_start(kqw_sb[:], kq_wT.transpose([1, 0, 2]))
            vw_sb = constp.tile([128, NDT, H], BF16, tag="vw")
            nc.sync.dma_start(vw_sb[:], v_wT.transpose([1, 0, 2]))
            for pre in range(1, XT_PRE):
                t = xtp.tile([128, NDT, S], BF16, tag="xt", name=f"xtp{pre}")
                nc.sync.dma_start(t[:], xT[pre].transpose([1, 0, 2]))
                xt_tiles[pre] = t

            # ---- persistent constants (batched loads) ----
            kqb_sb = constp.tile([128, 2 * H], F32, tag="kqb")
            nc.sync.dma_start(kqb_sb[:], kq_b[:])
            vb_sb = []
            for ht in range(NHT):
                t = constp.tile([128, 1], F32, tag=f"vb{ht}")
                nc.sync.dma_start(t[:], v_b2[ht])
                vb_sb.append(t)
            ab_sb = constp.tile([128, NST, H], BF16, tag="ab")
            nc.sync.dma_start(ab_sb[:], ab.transpose([1, 0, 2]))
            outb_sb = constp.tile([128, D], F32, tag="outb")
            nc.sync.dma_start(outb_sb[:], outb8[:])
            if not ln_trivial:
                lng_sb = constp.tile([128, H], F32, tag="lng")
                nc.sync.dma_start(lng_sb[:], lng[:])
                lnb_sb = constp.tile([128, H], F32, tag="lnb")
                nc.sync.dma_start(lnb_sb[:], lnb[:])
            onesb_sb = constp.tile([128, 128], BF16, tag="onesb")
            nc.sync.dma_start(onesb_sb[:], ones_b[:])
            eps_sb = constp.tile([128, 1], F32, tag="eps")
            nc.gpsimd.memset(eps_sb[:], LN_EPS)

            bigps = _pool("bigps", 3, space="PSUM")
            smallps = _pool("smallps", 3, space="PSUM")
            yps = _pool("yps", 2, space="PSUM")
            if True:
                # phase-8 state
                a2a2d = [a.rearrange("a b c d -> (a b) (c d)") for a in a2a_out]
                nrows = [8 * ln for lo, ln in CHUNKS]  # global rows per chunk
                ypsum = [None] * len(CHUNKS)
                ow_res = {}     # group -> resident ow tile

                def load_ow_group(g, resident):
                    if resident:
                        t = owresp.tile([128, P8G, D], BF16, tag=f"owr{g}",
                                        name=f"owr{g}")
                    else:
                        t = owstrp.tile([128, P8G, D], BF16, tag="owstr",
                                        name="owstr")
                    nc.sync.dma_start(
                        t[:], owT[g * P8G:(g + 1) * P8G].transpose([1, 0, 2]))
                    return t

                p8_emitted = [0] * len(CHUNKS)

                def emit_p8_group(p, g, ow_t):
                    nr = nrows[p]
                    if ypsum[p] is None:
                        ypsum[p] = yps.tile([128, D], F32, tag="yps",
                                            name=f"ypsum{p}")
                    pool = at0p if nr > 64 else at1p
                    at_t = pool.tile([128, P8G, nr], BF16, tag="at",
                                     name=f"at{p}_{g}")
                    nc.sync.dma_start_transpose(
                        at_t[:],
                        a2a2d[p][:, g * P8G * 128:(g + 1) * P8G * 128])
                    for i in range(P8G):
                        e = p8_emitted[p]
                        p8_emitted[p] = e + 1
                        nc.tensor.matmul(
                            ypsum[p][0:nr, :], at_t[:, i, :], ow_t[:, i, :],
                            start=(e == 0), stop=(e == NC_T - 1))

                stream_log = []  # (g, tile) in load order; last 2 live

                def get_ow(g):
                    if g < RES_G:
                        return ow_res[g]
                    for gg, h in stream_log[-2:]:
                        if gg == g:
                            return h
                    h = load_ow_group(g, resident=False)
                    stream_log.append((g, h))
                    return h

                def finish_chunk(p):
                    lo, ln = CHUNKS[p]
                    nr = nrows[p]
                    y_sb = ysbp.tile([128, D], F32, tag="ysb",
                                     name=f"ysb{p}")
                    nc.vector.tensor_add(y_sb[0:nr, :], ypsum[p][0:nr, :],
                                         outb_sb[0:nr, :])
                    nc.sync.dma_start(y_bounce[p][:], y_sb[0:nr, :])
                    nc.gpsimd.collective_compute(
                        "AllReduce", mybir.AluOpType.add,
                        replica_groups=[list(range(N_CORES))],
                        ins=[y_bounce[p].opt()], outs=[y_red[p].opt()])
                    dst = y_out.rearrange("(a e) d -> a e d",
                                          a=N_CORES)[:, lo:lo + ln]
                    nc.gpsimd.dma_start(dst, y_red[p][:])

                # ---- per-batch attention pipeline (ACT-table phase groups) ----
                pend = []            # deferred-GELU state per batch in group
                grp_tbl_insts = []   # this group's exp/ln ACT instructions
                prev_gelu = None     # last gelu instruction of previous group
                stat_pend = None     # previous batch's deferred LN-stat chain
                for b in range(NB):
                    if b + XT_PRE < NB:
                        t = xtp.tile([128, NDT, S], BF16, tag="xt",
                                     name=f"xtp{b + XT_PRE}")
                        nc.sync.dma_start(
                            t[:], xT[b + XT_PRE].transpose([1, 0, 2]))
                        xt_tiles[b + XT_PRE] = t
                    xt = xt_tiles.pop(b)

                    # vT[h, s] = sum_d v_wT[d, h] * xT[d, s]  (+v_b per-part)
                    vt_sb = []
                    for ht in range(NHT):
                        ps = bigps.tile([128, S], F32, tag="bigps")
                        for dt_ in range(NDT):
                            nc.tensor.matmul(
                                ps[:], vw_sb[:, dt_, ht * 128:(ht + 1) * 128],
                                xt[:, dt_, :],
                                start=(dt_ == 0), stop=(dt_ == NDT - 1))
                        t = vtp.tile([128, S], BF16, tag="vt")
                        nc.scalar.activation(t[:], ps[:], AF.Identity,
                                             bias=vb_sb[ht][:])
                        vt_sb.append(t)

                    # kq[s, j] = sum_d x[s, d] * [k_wT | q_wT][d, j]  (+bias)
                    kq_sb = []
                    for st in range(NST):
                        ps = bigps.tile([128, 2 * H], F32, tag="bigps")
                        for dt_ in range(NDT):
                            nc.tensor.matmul(
                                ps[:], xt[:, dt_, st * 128:(st + 1) * 128],
                                kqw_sb[:, dt_, :],
                                start=(dt_ == 0), stop=(dt_ == NDT - 1))
                        t = kqp.tile([128, 2 * H], BF16, tag="kq")
                        nc.vector.tensor_add(t[:], ps[:], kqb_sb[:])
                        kq_sb.append(t)

                    # scores[h, g] = sum_s k[s, h] q[s, g]; e = exp(scores/16)
                    e_sb = []
                    for ht in range(NHT):
                        sc = smallps.tile([128, H], F32, tag="smallps")
                        for st in range(NST):
                            nc.tensor.matmul(
                                sc[:], kq_sb[st][:, ht * 128:(ht + 1) * 128],
                                kq_sb[st][:, H:2 * H],
                                start=(st == 0), stop=(st == NST - 1))
                        t = ep.tile([128, H], BF16, tag="e")
                        ei = nc.scalar.activation(t[:], sc[:], AF.Exp,
                                                  scale=SCALE)
                        grp_tbl_insts.append(ei)
                        e_sb.append(t)

                    # softmax denom over h (partition dim) via all-ones
                    # matmul — every partition holds the full denom row, so
                    # the reciprocal runs wide and no broadcast is needed.
                    # Normalization is deferred until after the V @ E matmul
                    # to keep it off the tensor critical path.
                    sm = smallps.tile([128, H], F32, tag="smallps")
                    for ht in range(NHT):
                        nc.tensor.matmul(sm[:], onesb_sb[:], e_sb[ht][:],
                                         start=(ht == 0), stop=(ht == NHT - 1))
                    bc_sb = bcp.tile([128, H], F32, tag="bc")
                    nc.vector.reciprocal_approx_fast(bc_sb[:], sm[:])

                    # previous batch's LN-stat chain goes on the vector queue
                    # here, BEHIND this batch's kq bias-adds, so the adds
                    # (which gate the next scores matmul) aren't queued
                    # behind ~3us of stats.
                    if stat_pend is not None:
                        stat_pend()
                        stat_pend = None

                    # out5u[s, g] = sum_h vT[h, s] e[h, g]; normalize; +bias
                    eng = nc.gpsimd if GPS_TT else nc.vector
                    tl, rl, nl = [], [], []
                    for st in range(NST):
                        p5 = smallps.tile([128, H], F32, tag="smallps")
                        for ht in range(NHT):
                            nc.tensor.matmul(
                                p5[:], vt_sb[ht][:, st * 128:(st + 1) * 128],
                                e_sb[ht][:],
                                start=(ht == 0), stop=(ht == NHT - 1))
                        t1 = t1p.tile([128, H], BF16, tag="t1")
                        eng.tensor_mul(t1[:], p5[:], bc_sb[:])
                        t_sb = tp.tile([128, H], BF16, tag="t")
                        eng.tensor_add(t_sb[:], t1[:], ab_sb[:, st, :])
                        tl.append(t_sb)
                        rl.append([])
                        nl.append([])

                    def make_stat(tl=tl, rl=rl, nl=nl):
                        def stat():
                            for st in range(NST):
                                st6 = statp.tile([128, 6], F32, tag="st6")
                                nc.vector.bn_stats(st6[:], tl[st][:])
                                mv = statp.tile([128, 2], F32, tag="mv")
                                nc.vector.bn_aggr(mv[:], st6[:])
                                # rstd = (var+eps)^-0.5 = exp(-.5*ln(var+eps))
                                lnv = lnstatp.tile([128, 1], F32, tag="lnv")
                                li = nc.scalar.activation(
                                    lnv[:], mv[:, 1:2], AF.Ln, bias=eps_sb[:])
                                grp_tbl_insts.append(li)
                                rstd = lnstatp.tile([128, 1], F32, tag="rstd")
                                ri = nc.scalar.activation(
                                    rstd[:], lnv[:], AF.Exp, scale=-0.5)
                                grp_tbl_insts.append(ri)
                                nb_t = lnstatp.tile([128, 1], F32, tag="nb")
                                nc.vector.tensor_scalar(
                                    nb_t[:], mv[:, 0:1], rstd[:], -1.0,
                                    mybir.AluOpType.mult, mybir.AluOpType.mult)
                                rl[st] = rstd
                                nl[st] = nb_t
                        return stat
                    stat_pend = make_stat()
                    pend.append((b, tl, rl, nl))

                    # ---- deferred GELU pass for the finished group ----
                    if (b + 1) % G == 0:
                        stat_pend()
                        stat_pend = None
                        if prev_gelu is not None:
                            # keep ACT table phases disjoint across groups
                            for inst in grp_tbl_insts:
                                add_dep_helper(inst.ins, prev_gelu.ins,
                                               sync=False,
                                               reason="act-table grouping")
                        last_tbl = grp_tbl_insts[-1]
                        grp_tbl_insts = []
                        for pb_, tl, rl, nl in pend:
                            pc, ppb = _chunk_of(pb_)
                            act4 = actp.tile([128, NST, H], BF16, tag="act")
                            for st in range(NST):
                                if ln_trivial:
                                    gi = nc.scalar.activation(
                                        act4[:, st, :], tl[st][:], AF.Gelu,
                                        bias=nl[st][:], scale=rl[st][:])
                                else:
                                    nrm = tp.tile([128, H], F32, tag="nrm")
                                    nc.scalar.activation(
                                        nrm[:], tl[st][:], AF.Identity,
                                        bias=nl[st][:], scale=rl[st][:])
                                    nc.vector.tensor_mul(nrm[:], nrm[:],
                                                         lng_sb[:])
                                    nc.vector.tensor_add(nrm[:], nrm[:],
                                                         lnb_sb[:])
                                    gi = nc.scalar.activation(
                                        act4[:, st, :], nrm[:], AF.Gelu)
                                add_dep_helper(gi.ins, last_tbl.ins,
                                               sync=False,
                                               reason="act-table grouping")
                                prev_gelu = gi
                            # two stores per batch: row p, block st -> dst
                            # core 2*st + p//64, row p%64
                            dst = a2a_in[pc][:, ppb].rearrange(
                                "(st ph) r h -> ph r st h", st=NST, ph=2)
                            nc.gpsimd.dma_start(dst[0], act4[0:64])
                            nc.gpsimd.dma_start(dst[1], act4[64:128])
                        pend = []

                    # chunk AllToAlls as soon as their stores are queued
                    for pc, (lo, ln) in enumerate(CHUNKS):
                        if b == lo + ln - 1:
                            nc.gpsimd.collective_compute(
                                "AllToAll", mybir.AluOpType.bypass,
                                replica_groups=[list(range(N_CORES))],
                                ins=[a2a_in[pc].opt()],
                                outs=[a2a_out[pc].opt()])

                    # prefetch resident ow groups during mid-attention
                    if 10 <= b < 10 + RES_G:
                        g = b - 10
                        ow_res[g] = load_ow_group(g, resident=True)

                    # phase-8 passes 0/1 interleaved into late attention
                    P0 = {24: [0], 25: [1], 26: [2], 27: [3], 28: [4, 5],
                          29: [6, 7]}
                    P1 = {30: [0, 1, 2, 3], 31: [4, 5, 6, 7]}
                    for g in P0.get(b, ()):
                        emit_p8_group(0, g, get_ow(g))
                        if g == NP8G - 1:
                            finish_chunk(0)
                    for g in P1.get(b, ()):
                        emit_p8_group(1, g, get_ow(g))
                        if g == NP8G - 1:
                            finish_chunk(1)

                # ---- tail: last chunk's pass (+ AllReduce/copy) ----
                for g in range(NP8G):
                    emit_p8_group(2, g, get_ow(g))
                finish_chunk(2)

    nc.compile()
    return nc


_CACHE = {}


def _get_program(ln_trivial):
    if ln_trivial not in _CACHE:
        _CACHE[ln_trivial] = _build(ln_trivial)
    return _CACHE[ln_trivial]


def _prep_inputs(x, k_w, k_b, q_w, q_b, v_w, v_b, attn_bias, ln_g, ln_b,
                 out_w, out_b):
    ln_trivial = bool(np.all(ln_g == 1.0) and np.all(ln_b == 0.0))
    kq_wT = np.ascontiguousarray(
        np.concatenate([k_w.T, q_w.T], axis=1).astype(BBF16)
    ).reshape(NDT, 128, 2 * H)
    v_wT = np.ascontiguousarray(v_w.T.astype(BBF16)).reshape(NDT, 128, H)
    kq_b = np.ascontiguousarray(
        np.tile(np.concatenate([k_b, q_b])[None, :], (128, 1)))
    v_b2 = np.ascontiguousarray(v_b.reshape(NHT, 128, 1))
    ab = np.ascontiguousarray(attn_bias.astype(BBF16).reshape(NST, 128, H))
    outb8 = np.ascontiguousarray(np.tile((out_b / 8.0)[None, :], (128, 1)))
    owT_full = np.ascontiguousarray(out_w.T.astype(BBF16))  # [S*H, D]
    x16 = x.astype(BBF16)
    shared = dict(kq_wT=kq_wT, v_wT=v_wT, kq_b=kq_b, v_b2=v_b2, ab=ab,
                  outb8=outb8, ones_b=np.ones((128, 128), BBF16))
    if not ln_trivial:
        shared["lng"] = np.ascontiguousarray(np.tile(ln_g[None, :], (128, 1)))
        shared["lnb"] = np.ascontiguousarray(np.tile(ln_b[None, :], (128, 1)))
    in_maps = []
    for i in range(N_CORES):
        xi = np.ascontiguousarray(
            x16[i * NB:(i + 1) * NB].transpose(0, 2, 1)).reshape(
                NB, NDT, 128, S)
        owi = np.ascontiguousarray(
            owT_full[i * SLICE:(i + 1) * SLICE]).reshape(NC_T, 128, D)
        m = dict(shared)
        m["xT"] = xi
        m["owT"] = owi
        in_maps.append(m)
    return ln_trivial, in_maps


def kernel(**inputs):
    xs = {k: np.asarray(v, dtype=np.float32) for k, v in inputs.items()}
    ln_trivial, in_maps = _prep_inputs(
        xs["x"], xs["k_w"], xs["k_b"], xs["q_w"], xs["q_b"], xs["v_w"],
        xs["v_b"], xs["attn_bias"], xs["ln_g"], xs["ln_b"], xs["out_w"],
        xs["out_b"])
    nc = _get_program(ln_trivial)
    res = run_bass_kernel_spmd(nc, in_maps, core_ids=list(range(N_CORES)))
    y = res.results[0]["y"]  # post-AllReduce: identical on every core
    return y.reshape(B, 1, D).astype(np.float32)
